# revision 26
# baseline (speedup 1.0000x reference)
"""DOM pooling (segment mean+max over pulses, then linear projection) on 8 trn2 cores.

v2 strategy (vs baseline's dom-in-partition two-pass tensor_reduce, which is
DVE-bound at 1x mode):
  Layout: embed-in-partition. A window = 128 "dompair" columns; partition rows
  0-63 hold the 64 embed dims of the top dom, rows 64-127 the bottom dom.
  A dom's k pulse slots are k consecutive 128-column blocks. Data stored bf16
  (tolerance 2e-2; bf16 adds ~4e-3), halving HBM traffic.

  Per window group on device:
    - segment SUM + mean-scaling + projection fused on the TensorE: one
      accumulating matmul per slot into PSUM with per-class block-diag
      weights block_diag((W1/k).T, (W1/k).T).
    - segment MAX on DVE as a pairwise tensor_max tree (2x_1p mode in bf16,
      one instruction per level via strided views) instead of 1x tensor_reduce.
    - max projection: one more accumulating matmul with block_diag(W2.T, W2.T)
      into the same PSUM -> full concat projection accumulated in place.
    - ACT adds bias during PSUM->SBUF copy; DMA writes [128, cols] f32 out.

  Doms are bucketed by exact pulse count k (class) and dealt round-robin over
  the 8 cores (identical structure per core -> one SPMD NEFF). Full windows
  hold one class (k uniform, 1/k folded into weights). Per-class leftovers are
  packed into shared "ragged" windows (sorted by k, slots padded to the group
  max with dup-of-slot-0 which is max-neutral, plus one host-built correction
  slot -padcnt*x0 so the PE sum stays exact); the per-dom 1/k is applied via
  an elementwise recip table multiply on DVE, then max-proj + bias added with
  one scalar_tensor_tensor.
"""
import os
import sys

import numpy as np

for _p in ("/opt/trn_rl_repo",):
    if _p not in sys.path:
        sys.path.append(_p)

import ml_dtypes
from concourse import bacc
import concourse.mybir as mybir
import concourse.tile as tile
from concourse.bass_utils import run_bass_kernel_spmd

NCORES = 8
D = 64
FP32 = mybir.dt.float32
BF16 = mybir.dt.float16
BF = np.float16

GS_BUDGET = 128   # max g*S per group (SBUF in-tile = [128, GS_BUDGET*128] bf16)
GMAX = 16        # max windows per group

last_exec_ns = None  # set when KERNEL_TRACE=1


def _install_trace_hook():
    """Best-effort: synthesize antenv.axon_hooks + NTFF hook so
    run_bass_kernel_spmd(trace=True) works under axon."""
    import types

    try:
        if "antenv.axon_hooks" in sys.modules:
            return True
        import antenv

        mod = types.ModuleType("antenv.axon_hooks")
        mod._hook = None
        mod.set_axon_ntff_profile_hook = lambda h: setattr(mod, "_hook", h)
        mod.get_axon_ntff_profile_hook = lambda: mod._hook
        sys.modules["antenv.axon_hooks"] = mod
        antenv.axon_hooks = mod
        from trn_agent_boot.trn_boot import _ntff_profile_via_ctypes

        hook = _ntff_profile_via_ctypes("/opt/axon/libaxon_pjrt.so")
        if hook is not None:
            mod._hook = hook
        return hook is not None
    except Exception:
        return False


def _plan(counts):
    """Shared (all-core) window/group plan derived from global counts.

    Returns dict with:
      ks: ascending class pulse-counts
      cls: {k: (n_k, m_k, fw_k, r_k, rag_off_k, fullwin0_k)}
      full_groups: [(k, jcls, win0, g, S, base)]
      rag_groups:  [(win0_r, g, S, base)]      win0_r is ragged-window index
      nfullwin, RW, nwin, s_elems, kwin (per ragged window), karr (rag layout)
    """
    nd = counts.shape[0]
    kmax = int(counts.max()) if nd else 0
    n_k = np.bincount(counts, minlength=kmax + 1)
    ks = [k for k in range(1, kmax + 1) if n_k[k] > 0]

    cls = {}
    rag_off = 0
    fullwin = 0
    for k in ks:
        n = int(n_k[k])
        m = -(-n // NCORES)
        fw = m // 256
        r = m % 256
        cls[k] = [n, m, fw, r, rag_off, fullwin]
        rag_off += r
        fullwin += fw
    R = rag_off
    RW = -(-R // 256) if R else 0

    # ragged layout: class k occupies rpos [rag_off_k, rag_off_k + r_k)
    karr = np.zeros(RW * 256, np.int32)
    for k in ks:
        _, _, _, r, ro, _ = cls[k]
        if r:
            karr[ro : ro + r] = k
    kwin = karr.reshape(RW, 256).max(axis=1) if RW else np.zeros(0, np.int32)

    # full groups
    full_groups = []
    base = 0
    for jcls, k in enumerate(ks):
        _, _, fw, _, _, fullwin0 = cls[k]
        if fw == 0:
            continue
        g_cap = min(GMAX, max(1, GS_BUDGET // k))
        w = 0
        while w < fw:
            g = min(g_cap, fw - w)
            full_groups.append((k, jcls, fullwin0 + w, g, k, base))
            base += g * 128 * k * 128
            w += g

    # ragged groups (greedy, S = max kw in group + 1; cap pad growth so a
    # group never stores more than ~1.15x the per-window-exact slot count)
    rag_groups = []
    rw = 0
    while rw < RW:
        g = 1
        S = int(kwin[rw]) + 1
        exact = S
        while rw + g < RW:
            S2 = max(S, int(kwin[rw + g]) + 1)
            exact2 = exact + int(kwin[rw + g]) + 1
            if (
                (g + 1) * S2 > GS_BUDGET
                or g + 1 > GMAX
                or (g + 1) * S2 > float(os.environ.get("KERNEL_RAG_PAD_CAP", "9.9")) * exact2
            ):
                break
            S = S2
            exact = exact2
            g += 1
        rag_groups.append((rw, g, S, base))
        base += g * 128 * S * 128
        rw += g

    # Select which classes/groups compute their segment-sum on the DVE
    # (add-tree) instead of the PE, to balance the two engines. Units with
    # the worst PE efficiency (ceil(S/4)*4/S) are offloaded first, up to
    # DVE_SUM_FRAC of the total slot elements.
    frac = float(os.environ.get("KERNEL_DVE_FRAC", "0.3"))
    units = []
    for k in ks:
        if cls[k][2]:
            el = sum(g * S * 128 * 128 for kk, _, _, g, S, _ in full_groups if kk == k)
            units.append((-(-k // 4) * 4 / k, el, ("cls", k)))
    for gi, (_, g, S, _) in enumerate(rag_groups):
        el = g * S * 128 * 128
        units.append((-(-S // 4) * 4 / S, el, ("rag", gi)))
    units.sort(key=lambda u: -u[0])
    target = frac * max(base, 1)
    acc = 0
    dve_cls = set()
    dve_rag = set()
    for _, el, (kind, ident) in units:
        if acc >= target:
            break
        acc += el
        (dve_cls if kind == "cls" else dve_rag).add(ident)

    max_gs = max(
        [g * S for _, _, _, g, S, _ in full_groups]
        + [g * S for _, g, S, _ in rag_groups]
        + [1]
    )
    max_g = max(
        [g for _, _, _, g, _, _ in full_groups]
        + [g for _, g, _, _ in rag_groups]
        + [1]
    )
    return dict(
        ks=ks, cls=cls, full_groups=full_groups, rag_groups=rag_groups,
        nfullwin=fullwin, RW=RW, nwin=fullwin + RW, s_elems=max(base, 1),
        kwin=kwin, karr=karr, max_gs=max_gs, max_g=max_g,
        dve_cls=dve_cls, dve_rag=dve_rag,
    )


def _build_nc(plan, nb):
    ks = plan["ks"]
    ncols = plan["nwin"] * 128
    nrw = plan["RW"]
    jW1un = len(ks)       # unscaled BD(W1.T) for ragged sums
    jW2 = len(ks) + 1     # BD(W2.T) for max projection

    nc = bacc.Bacc(None)
    slots_t = nc.dram_tensor("slots", [plan["s_elems"]], BF16, kind="ExternalInput")
    wts_t = nc.dram_tensor("wts", [nb * 128, 128], BF16, kind="ExternalInput")
    b_t = nc.dram_tensor("b", [128, 1], FP32, kind="ExternalInput")
    if nrw:
        rt_t = nc.dram_tensor("rt", [128, nrw * 128], BF16, kind="ExternalInput")
    out_t = nc.dram_tensor("out", [128, ncols], BF16, kind="ExternalOutput")

    INF = plan["max_gs"] * 128  # in-tile free elems
    MAXG = plan["max_g"]

    with tile.TileContext(nc) as tc:
        with (
            tc.tile_pool(name="const", bufs=1) as constp,
            tc.tile_pool(name="inp", bufs=3) as inp,
            tc.tile_pool(name="treep", bufs=2) as treep,
            tc.tile_pool(name="tmpp", bufs=2) as tmpp,
            tc.tile_pool(name="outp", bufs=2) as outp,
            tc.tile_pool(name="psA", bufs=6, space="PSUM") as psA,
            tc.tile_pool(name="psB", bufs=2, space="PSUM") as psB,
        ):
            wt_sb = constp.tile([128, nb * 128], BF16)
            nc.sync.dma_start(
                wt_sb[:].rearrange("p (j m) -> p j m", m=128),
                wts_t[:, :].rearrange("(j p) m -> p j m", p=128),
            )
            b_sb = constp.tile([128, 1], FP32)
            nc.sync.dma_start(b_sb[:], b_t[:])
            if nrw:
                rt_sb = constp.tile([128, nrw * 128], BF16)
                nc.sync.dma_start(rt_sb[:], rt_t[:, :])

            def lhs(j):
                return wt_sb[:, j * 128 : (j + 1) * 128]

            def tree(v, g, n, op, tag):
                """DVE pairwise reduction tree over the slot axis of a 4D
                view [p, gg, s, d]; one tensor_tensor per level (2x_1p in
                fp16). Returns [p, gg, d]."""
                cur = v
                carries = []
                lvl = 0
                while n > 1:
                    h = n // 2
                    if n % 2:
                        carries.append(cur[:, :, n - 1, :])
                    lvl += 1
                    tnew = treep.tile(
                        [128, max(INF // (2 ** lvl), 128)], BF16,
                        tag=f"{tag}{lvl}",
                    )
                    ov = tnew[:, : g * h * 128].rearrange(
                        "p (gg s d) -> p gg s d", gg=g, s=h
                    )
                    nc.vector.tensor_tensor(
                        out=ov, in0=cur[:, :, 0 : 2 * h : 2, :],
                        in1=cur[:, :, 1 : 2 * h : 2, :], op=op,
                    )
                    cur = ov
                    n = h
                m3 = cur[:, :, 0, :]  # [p, gg, d]
                for cv in carries:
                    tcar = treep.tile([128, MAXG * 128], BF16, tag=f"{tag}c")
                    c3 = tcar[:, : g * 128].rearrange("p (gg d) -> p gg d", gg=g)
                    nc.vector.tensor_tensor(out=c3, in0=m3, in1=cv, op=op)
                    m3 = c3
                return m3

            def do_group(S, g, base, col0, jsum, ragged, dve_sum, rtc0=0):
                F = g * S * 128
                in_t = inp.tile([128, INF], BF16, tag="in")
                nc.sync.dma_start(
                    in_t[:, :F].rearrange("p (gg f) -> p gg f", gg=g),
                    slots_t[base : base + 128 * F].rearrange(
                        "(gg p f) -> p gg f", gg=g, p=128
                    ),
                )
                v = in_t[:, :F].rearrange("p (gg s d) -> p gg s d", gg=g, s=S)

                # max tree first: it gates the PE's W2 projection, so it
                # should land before the sum tree on the in-order DVE.
                S_tree = S - 1 if ragged else S
                m3 = tree(v, g, S_tree, mybir.AluOpType.max, "m")
                # segment sum: either a DVE add-tree (then one projection
                # matmul per chunk), or slot-fused matmuls on the PE (stride-0
                # out AP accumulates 4 slots per instruction).
                sum3 = tree(v, g, S, mybir.AluOpType.add, "s") if dve_sum else None

                chunks = []
                for c0 in range(0, g, 4):
                    cw = min(4, g - c0)
                    ps = psA.tile([128, 512], FP32, space="PSUM", tag="ps")
                    if dve_sum:
                        if not ragged:
                            # W2*max first: the max tree lands on DVE before
                            # the sum tree, so this unblocks the PE earlier.
                            nc.tensor.matmul(
                                ps[:, : cw * 128],
                                lhsT=lhs(jW2),
                                rhs=m3[:, c0 : c0 + cw, :],
                                start=True,
                                stop=False,
                            )
                        nc.tensor.matmul(
                            ps[:, : cw * 128],
                            lhsT=lhs(jsum),
                            rhs=sum3[:, c0 : c0 + cw, :],
                            start=bool(ragged),
                            stop=True,
                        )
                    else:
                        for ci in range(cw):
                            # ISA: matmul rhs <= 512 elems/partition, APs 3D
                            # max. start=True only on the bank's FIRST matmul:
                            # start resets the bank-wide has-written bits, so
                            # a later start would make subsequent writes
                            # overwrite earlier windows' accumulated sums.
                            for s0 in range(0, S, 4):
                                sr = min(4, S - s0)
                                out_b = (
                                    ps[:, ci * 128 : (ci + 1) * 128]
                                    .rearrange("p (one d) -> p one d", one=1)
                                    .broadcast_to([128, sr, 128])
                                )
                                nc.tensor.matmul(
                                    out_b,
                                    lhsT=lhs(jsum),
                                    rhs=v[:, c0 + ci, s0 : s0 + sr, :],
                                    start=(ci == 0 and s0 == 0),
                                    stop=bool(ragged)
                                    and ci == cw - 1
                                    and s0 + sr >= S,
                                )
                    chunks.append((c0, cw, ps))

                # ---- PE: max projection; ACT/DVE: bias + writeout -------
                out_sb = outp.tile([128, MAXG * 128], BF16, tag="out")
                for c0, cw, ps in chunks:
                    rhs_m = m3[:, c0 : c0 + cw, :]
                    oc = out_sb[:, c0 * 128 : (c0 + cw) * 128]
                    if not ragged:
                        if not dve_sum:  # dve_sum emitted W2 in the loop above
                            nc.tensor.matmul(
                                ps[:, : cw * 128], lhsT=lhs(jW2), rhs=rhs_m,
                                start=False, stop=True,
                            )
                        nc.scalar.activation(
                            oc, ps[:, : cw * 128],
                            mybir.ActivationFunctionType.Identity,
                            bias=b_sb[:, :1],
                        )
                    else:
                        psb = psB.tile([128, 512], FP32, space="PSUM", tag="psb")
                        nc.tensor.matmul(
                            psb[:, : cw * 128], lhsT=lhs(jW2), rhs=rhs_m,
                            start=True, stop=True,
                        )
                        tmp = tmpp.tile([128, MAXG * 128], BF16, tag="tmp")
                        tc_ = tmp[:, : cw * 128]
                        nc.vector.tensor_mul(
                            tc_, ps[:, : cw * 128],
                            rt_sb[:, rtc0 + c0 * 128 : rtc0 + (c0 + cw) * 128],
                        )
                        nc.vector.scalar_tensor_tensor(
                            out=oc, in0=psb[:, : cw * 128], scalar=b_sb[:, :1],
                            in1=tc_, op0=mybir.AluOpType.add,
                            op1=mybir.AluOpType.add,
                        )
                nc.scalar.dma_start(
                    out_t[:, col0 : col0 + g * 128], out_sb[:, : g * 128]
                )

            # Interleave ragged groups among the full groups (never last):
            # ragged groups have longer per-group dependency chains, so
            # placing one at the program tail adds un-overlapped latency.
            jobs = [
                (
                    "full",
                    (S, g, base, win0 * 128, jcls),
                    k in plan["dve_cls"],
                )
                for k, jcls, win0, g, S, base in plan["full_groups"]
            ]
            rjobs = [
                (
                    "rag",
                    (
                        S, g, base, (plan["nfullwin"] + win0_r) * 128,
                        jW1un, win0_r * 128,
                    ),
                    gi in plan["dve_rag"],
                )
                for gi, (win0_r, g, S, base) in enumerate(plan["rag_groups"])
            ]
            ordered = jobs + rjobs
            if os.environ.get("KERNEL_INTERLEAVE", "0") == "1" and jobs and rjobs:
                nf = len(jobs)
                step = max(nf // (len(rjobs) + 1), 1)
                ordered = []
                ri = 0
                for i, jb in enumerate(jobs):
                    ordered.append(jb)
                    if ri < len(rjobs) and (i + 1) % step == 0 and i < nf - 2:
                        ordered.append(rjobs[ri])
                        ri += 1
                ins = max(len(ordered) - 2, 0)
                ordered[ins:ins] = rjobs[ri:]
            for kind, args, dvs in ordered:
                if kind == "full":
                    S, g, base, col0, jcls = args
                    do_group(S, g, base, col0, jcls, ragged=False, dve_sum=dvs)
                else:
                    S, g, base, col0, jsum, rtc0 = args
                    do_group(
                        S, g, base, col0, jsum, ragged=True, dve_sum=dvs,
                        rtc0=rtc0,
                    )
    nc.finalize()
    return nc


def kernel(pulse_embeddings, pulse_to_dom_idx, num_doms, proj_w, proj_b):
    global last_exec_ns

    E = np.asarray(pulse_embeddings, dtype=np.float32)
    idx = np.asarray(pulse_to_dom_idx).astype(np.int64)
    nd = int(num_doms)
    W = np.asarray(proj_w, dtype=np.float32)   # (D, 2D)
    b = np.asarray(proj_b, dtype=np.float32)   # (D,)

    counts = np.bincount(idx, minlength=nd).astype(np.int64)
    plan = _plan(counts)
    ks = plan["ks"]
    cls = plan["cls"]
    nb = len(ks) + 2

    # ---- dom assignment (vectorized, shared structure) -------------------
    order = np.argsort(counts, kind="stable")
    n0 = int((counts == 0).sum())
    dom_core = np.full(nd, -1, np.int32)
    dom_win = np.zeros(nd, np.int64)
    dom_half = np.zeros(nd, np.int32)
    dom_dp = np.zeros(nd, np.int32)
    nfullwin = plan["nfullwin"]

    off = n0
    for k in ks:
        n, m, fw, r, rag_off_k, fullwin0 = cls[k]
        doms = order[off : off + n]
        off += n
        core = (np.arange(n) % NCORES).astype(np.int32)
        pos = (np.arange(n) // NCORES).astype(np.int64)
        isfull = pos < fw * 256
        rpos = rag_off_k + (pos - fw * 256)
        q = np.where(isfull, pos % 256, rpos % 256)
        win = np.where(isfull, fullwin0 + pos // 256, nfullwin + rpos // 256)
        dom_core[doms] = core
        dom_win[doms] = win
        dom_half[doms] = (q // 128).astype(np.int32)
        dom_dp[doms] = (q % 128).astype(np.int32)

    # ---- pulse permutation: sort by (core, window, half, dp) -------------
    dkey = (
        dom_core.astype(np.int64) * (1 << 40)
        + dom_win * 256
        + dom_half * 128
        + dom_dp
    )
    pkey = dkey[idx]
    perm = np.argsort(pkey, kind="stable")
    core_pulse_counts = np.bincount(dom_core[idx], minlength=NCORES)
    core_splits = np.concatenate([[0], np.cumsum(core_pulse_counts)])

    # ---- host-side slot buffer packing -----------------------------------
    E16 = E.astype(BF)
    s_elems = plan["s_elems"]
    RW = plan["RW"]

    # ragged group lookup per ragged window
    if RW:
        grp_of_rw = np.zeros(RW, np.int32)
        for gi, (win0_r, g, S, base) in enumerate(plan["rag_groups"]):
            grp_of_rw[win0_r : win0_r + g] = gi

    # full groups of each class, in order
    fg_of_k = {}
    for fg in plan["full_groups"]:
        fg_of_k.setdefault(fg[0], []).append(fg)

    e_idx = np.arange(D)
    bufs = []
    for c in range(NCORES):
        buf = np.zeros(s_elems, BF)
        pc = perm[core_splits[c] : core_splits[c + 1]]
        p = 0
        # full regions, class ascending
        for k in ks:
            n, m, fw, r, rag_off_k, fullwin0 = cls[k]
            nreal_c = n // NCORES + (1 if c < n % NCORES else 0)
            n_full = min(nreal_c, fw * 256)
            if fw == 0:
                continue
            if n_full == 0:
                continue
            rows = pc[p : p + n_full * k].reshape(n_full, k)
            p += n_full * k
            A = E16[rows]  # (n_full, k, 64)
            for (_, _, win0, g, S, base) in fg_of_k[k]:
                lo = (win0 - fullwin0) * 256
                hi = min(n_full, lo + g * 256)
                if hi <= lo:
                    continue
                T = buf[base : base + g * 128 * k * 128].reshape(g, 128, k, 128)
                nw_full = (hi - lo) // 256
                if nw_full:
                    B = (
                        A[lo : lo + nw_full * 256]
                        .reshape(nw_full, 2, 128, k, D)
                        .transpose(0, 1, 4, 3, 2)
                        .reshape(nw_full, 128, k, 128)
                    )
                    T[:nw_full] = B
                rem = (hi - lo) - nw_full * 256
                if rem:
                    Ar = A[lo + nw_full * 256 : hi]  # (rem, k, 64)
                    j = np.arange(rem)
                    h = (j // 128).astype(np.int32)
                    dp = (j % 128).astype(np.int32)
                    T[
                        nw_full,
                        (64 * h)[:, None, None] + e_idx[None, None, :],
                        np.arange(k)[None, :, None],
                        dp[:, None, None],
                    ] = Ar
        # ragged region, class ascending
        for k in ks:
            n, m, fw, r, rag_off_k, fullwin0 = cls[k]
            if r == 0:
                continue
            nreal_c = n // NCORES + (1 if c < n % NCORES else 0)
            n_full = min(nreal_c, fw * 256)
            n_rag = nreal_c - n_full
            if n_rag == 0:
                continue
            rows = pc[p : p + n_rag * k].reshape(n_rag, k)
            p += n_rag * k
            A = E16[rows]  # (n_rag, k, 64)
            rpos = rag_off_k + np.arange(n_rag)
            rw = rpos // 256
            grp = grp_of_rw[rw]
            bnds = np.concatenate(
                [[0], np.nonzero(np.diff(grp))[0] + 1, [n_rag]]
            )
            for bi in range(len(bnds) - 1):
                j0, j1 = int(bnds[bi]), int(bnds[bi + 1])
                gi = int(grp[j0])
                win0_r, g, S, base = plan["rag_groups"][gi]
                Ac = A[j0:j1]
                nn = j1 - j0
                Bf = np.empty((nn, S, D), BF)
                Bf[:, :k] = Ac
                padcnt = (S - 1) - k
                if padcnt > 0:
                    Bf[:, k : S - 1] = Ac[:, :1]
                Bf[:, S - 1] = (
                    -float(padcnt) * Ac[:, 0].astype(np.float32)
                ).astype(BF)
                T = buf[base : base + g * 128 * S * 128].reshape(g, 128, S, 128)
                rw_l = (rw[j0:j1] - win0_r).astype(np.int64)
                h = ((rpos[j0:j1] % 256) // 128).astype(np.int64)
                dp = (rpos[j0:j1] % 128).astype(np.int64)
                T[
                    rw_l[:, None, None],
                    (64 * h)[:, None, None] + e_idx[None, None, :],
                    np.arange(S)[None, :, None],
                    dp[:, None, None],
                ] = Bf
        bufs.append(buf)

    # ---- weights / bias / recip table ------------------------------------
    W1t = np.ascontiguousarray(W[:, :D].T)   # (64, 64) = W1.T
    W2t = np.ascontiguousarray(W[:, D:].T)
    wts = np.zeros((nb * 128, 128), np.float32)
    for j, k in enumerate(ks):
        blk = W1t / np.float32(k)
        wts[j * 128 : j * 128 + 64, :64] = blk
        wts[j * 128 + 64 : (j + 1) * 128, 64:] = blk
    j = len(ks)
    wts[j * 128 : j * 128 + 64, :64] = W1t
    wts[j * 128 + 64 : (j + 1) * 128, 64:] = W1t
    j = len(ks) + 1
    wts[j * 128 : j * 128 + 64, :64] = W2t
    wts[j * 128 + 64 : (j + 1) * 128, 64:] = W2t
    wts16 = wts.astype(BF)
    b2 = np.concatenate([b, b]).reshape(128, 1).astype(np.float32)

    rt = None
    if RW:
        karr = plan["karr"].astype(np.float64)
        recip = np.where(karr > 0, 1.0 / np.maximum(karr, 1), 0.0).astype(BF)
        rec = recip.reshape(RW, 2, 128)
        rt = np.zeros((128, RW * 128), BF)
        rt4 = rt.reshape(128, RW, 128)
        rt4[0:64] = rec[:, 0, :][None]
        rt4[64:128] = rec[:, 1, :][None]

    # ---- optional numpy emulation of the device program (KERNEL_SIM=1) ----
    if os.environ.get("KERNEL_SIM", "0") == "1":
        outs = np.stack(
            [_simulate(plan, bufs[c], wts, b2, rt, len(ks)) for c in range(NCORES)]
        )
        last_exec_ns = None
        return _unpermute(
            outs, plan, dom_core, dom_win, dom_half, dom_dp, nd, n0, b
        )

    # ---- device -----------------------------------------------------------
    if os.environ.get("KERNEL_LDW_OPT", "1") == "1":
        try:
            from concourse.compiler_utils import (
                get_compiler_flags,
                set_compiler_flags,
            )

            flags = [
                f.replace("--enable-ldw-opt=false", "--enable-ldw-opt=true")
                for f in get_compiler_flags()
            ]
            set_compiler_flags(flags)
        except Exception:
            pass
    nc = _build_nc(plan, nb)
    in_maps = []
    for c in range(NCORES):
        m = {"slots": bufs[c], "wts": wts16, "b": b2}
        if RW:
            m["rt"] = rt
        in_maps.append(m)
    trace = os.environ.get("KERNEL_TRACE", "0") == "1"
    kw_ = {}
    if trace and _install_trace_hook():
        import tempfile

        kw_ = dict(trace=True, tmpdir=tempfile.mkdtemp(prefix="kernel_trace_"))
    res = run_bass_kernel_spmd(nc, in_maps, core_ids=list(range(NCORES)), **kw_)
    last_exec_ns = res.exec_time_ns

    # ---- host-side unpermute ----------------------------------------------
    outs = np.stack(
        [np.asarray(res.results[c]["out"], dtype=np.float32) for c in range(NCORES)]
    )  # (8,128,ncols)
    return _unpermute(outs, plan, dom_core, dom_win, dom_half, dom_dp, nd, n0, b)


def _unpermute(outs, plan, dom_core, dom_win, dom_half, dom_dp, nd, n0, b):
    ncols = plan["nwin"] * 128
    outs_t = outs.transpose(0, 2, 1).reshape(NCORES, ncols, 2, D)
    full = np.empty((nd, D), np.float32)
    real = dom_core >= 0
    col = dom_win * 128 + dom_dp
    full[real] = outs_t[dom_core[real], col[real], dom_half[real]]
    if n0:
        full[~real] = b
    return full


def _simulate(plan, buf, wts, b2, rt, ncls):
    """Numpy emulation of the device program for one core (fp32 math)."""
    ncols = plan["nwin"] * 128
    out = np.zeros((128, ncols), np.float32)
    jW1un, jW2 = ncls, ncls + 1

    def blkT(j):
        return wts[j * 128 : (j + 1) * 128].astype(np.float32).T  # (128out,128in)

    for k, jcls, win0, g, S, base in plan["full_groups"]:
        T = buf[base : base + g * 128 * S * 128].reshape(g, 128, S, 128)
        Tf = T.astype(np.float32)
        ssum = Tf.sum(axis=2)          # (g, 128, 128)
        smax = Tf.max(axis=2)          # bf16-exact max
        for w in range(g):
            ps = blkT(jcls) @ ssum[w] + blkT(jW2) @ smax[w]
            out[:, (win0 + w) * 128 : (win0 + w + 1) * 128] = ps + b2
    for win0_r, g, S, base in plan["rag_groups"]:
        T = buf[base : base + g * 128 * S * 128].reshape(g, 128, S, 128)
        Tf = T.astype(np.float32)
        ssum = Tf.sum(axis=2)
        smax = Tf[:, :, : S - 1, :].max(axis=2)
        for w in range(g):
            rw = win0_r + w
            mean = (blkT(jW1un) @ ssum[w]) * rt[:, rw * 128 : (rw + 1) * 128]
            ps = mean + blkT(jW2) @ smax[w]
            out[:, (plan["nfullwin"] + rw) * 128 : (plan["nfullwin"] + rw + 1) * 128] = (
                ps + b2
            )
    return out


# revision 27
# speedup vs baseline: 1.1020x; 1.1020x over previous
"""DOM pooling (segment mean+max over pulses, then linear projection) on 8 trn2 cores.

v2 strategy (vs baseline's dom-in-partition two-pass tensor_reduce, which is
DVE-bound at 1x mode):
  Layout: embed-in-partition. A window = 128 "dompair" columns; partition rows
  0-63 hold the 64 embed dims of the top dom, rows 64-127 the bottom dom.
  A dom's k pulse slots are k consecutive 128-column blocks. Data stored bf16
  (tolerance 2e-2; bf16 adds ~4e-3), halving HBM traffic.

  Per window group on device:
    - segment SUM + mean-scaling + projection fused on the TensorE: one
      accumulating matmul per slot into PSUM with per-class block-diag
      weights block_diag((W1/k).T, (W1/k).T).
    - segment MAX on DVE as a pairwise tensor_max tree (2x_1p mode in bf16,
      one instruction per level via strided views) instead of 1x tensor_reduce.
    - max projection: one more accumulating matmul with block_diag(W2.T, W2.T)
      into the same PSUM -> full concat projection accumulated in place.
    - ACT adds bias during PSUM->SBUF copy; DMA writes [128, cols] f32 out.

  Doms are bucketed by exact pulse count k (class) and dealt round-robin over
  the 8 cores (identical structure per core -> one SPMD NEFF). Full windows
  hold one class (k uniform, 1/k folded into weights). Per-class leftovers are
  packed into shared "ragged" windows (sorted by k, slots padded to the group
  max with dup-of-slot-0 which is max-neutral, plus one host-built correction
  slot -padcnt*x0 so the PE sum stays exact); the per-dom 1/k is applied via
  an elementwise recip table multiply on DVE, then max-proj + bias added with
  one scalar_tensor_tensor.
"""
import os
import sys

import numpy as np

for _p in ("/opt/trn_rl_repo",):
    if _p not in sys.path:
        sys.path.append(_p)

import ml_dtypes
from concourse import bacc
import concourse.mybir as mybir
import concourse.tile as tile
from concourse.bass_utils import run_bass_kernel_spmd

NCORES = 8
D = 64
FP32 = mybir.dt.float32
BF16 = mybir.dt.float16
BF = np.float16

GS_BUDGET = 112   # max g*S per group (SBUF in-tile = [128, GS_BUDGET*128] bf16)
GMAX = 14        # max windows per group

last_exec_ns = None  # set when KERNEL_TRACE=1


def _install_trace_hook():
    """Best-effort: synthesize antenv.axon_hooks + NTFF hook so
    run_bass_kernel_spmd(trace=True) works under axon."""
    import types

    try:
        if "antenv.axon_hooks" in sys.modules:
            return True
        import antenv

        mod = types.ModuleType("antenv.axon_hooks")
        mod._hook = None
        mod.set_axon_ntff_profile_hook = lambda h: setattr(mod, "_hook", h)
        mod.get_axon_ntff_profile_hook = lambda: mod._hook
        sys.modules["antenv.axon_hooks"] = mod
        antenv.axon_hooks = mod
        from trn_agent_boot.trn_boot import _ntff_profile_via_ctypes

        hook = _ntff_profile_via_ctypes("/opt/axon/libaxon_pjrt.so")
        if hook is not None:
            mod._hook = hook
        return hook is not None
    except Exception:
        return False


def _plan(counts):
    """Shared (all-core) window/group plan derived from global counts.

    Returns dict with:
      ks: ascending class pulse-counts
      cls: {k: (n_k, m_k, fw_k, r_k, rag_off_k, fullwin0_k)}
      full_groups: [(k, jcls, win0, g, S, base)]
      rag_groups:  [(win0_r, g, S, base)]      win0_r is ragged-window index
      nfullwin, RW, nwin, s_elems, kwin (per ragged window), karr (rag layout)
    """
    nd = counts.shape[0]
    kmax = int(counts.max()) if nd else 0
    n_k = np.bincount(counts, minlength=kmax + 1)
    ks = [k for k in range(1, kmax + 1) if n_k[k] > 0]

    cls = {}
    rag_off = 0
    fullwin = 0
    for k in ks:
        n = int(n_k[k])
        m = -(-n // NCORES)
        fw = m // 256
        r = m % 256
        cls[k] = [n, m, fw, r, rag_off, fullwin]
        rag_off += r
        fullwin += fw
    R = rag_off
    RW = -(-R // 256) if R else 0

    # ragged layout: class k occupies rpos [rag_off_k, rag_off_k + r_k)
    karr = np.zeros(RW * 256, np.int32)
    for k in ks:
        _, _, _, r, ro, _ = cls[k]
        if r:
            karr[ro : ro + r] = k
    kwin = karr.reshape(RW, 256).max(axis=1) if RW else np.zeros(0, np.int32)

    # full groups
    full_groups = []
    base = 0
    for jcls, k in enumerate(ks):
        _, _, fw, _, _, fullwin0 = cls[k]
        if fw == 0:
            continue
        g_cap = min(GMAX, max(1, GS_BUDGET // k))
        w = 0
        while w < fw:
            g = min(g_cap, fw - w)
            full_groups.append((k, jcls, fullwin0 + w, g, k, base))
            base += g * 128 * k * 128
            w += g

    # ragged groups (greedy, S = max kw in group + 1; cap pad growth so a
    # group never stores more than ~1.15x the per-window-exact slot count)
    rag_groups = []
    rw = 0
    while rw < RW:
        g = 1
        S = int(kwin[rw]) + 1
        exact = S
        while rw + g < RW:
            S2 = max(S, int(kwin[rw + g]) + 1)
            exact2 = exact + int(kwin[rw + g]) + 1
            if (
                (g + 1) * S2 > GS_BUDGET
                or g + 1 > GMAX
                or (g + 1) * S2 > float(os.environ.get("KERNEL_RAG_PAD_CAP", "9.9")) * exact2
            ):
                break
            S = S2
            exact = exact2
            g += 1
        rag_groups.append((rw, g, S, base))
        base += g * 128 * S * 128
        rw += g

    # Select which classes/groups compute their segment-sum on the DVE
    # (add-tree) instead of the PE, to balance the two engines. Units with
    # the worst PE efficiency (ceil(S/4)*4/S) are offloaded first, up to
    # DVE_SUM_FRAC of the total slot elements.
    frac = float(os.environ.get("KERNEL_DVE_FRAC", "0.3"))
    units = []
    for k in ks:
        if cls[k][2]:
            el = sum(g * S * 128 * 128 for kk, _, _, g, S, _ in full_groups if kk == k)
            units.append((-(-k // 4) * 4 / k, el, ("cls", k)))
    for gi, (_, g, S, _) in enumerate(rag_groups):
        el = g * S * 128 * 128
        units.append((-(-S // 4) * 4 / S, el, ("rag", gi)))
    units.sort(key=lambda u: -u[0])
    target = frac * max(base, 1)
    acc = 0
    dve_cls = set()
    dve_rag = set()
    for _, el, (kind, ident) in units:
        if acc >= target:
            break
        acc += el
        (dve_cls if kind == "cls" else dve_rag).add(ident)

    max_gs = max(
        [g * S for _, _, _, g, S, _ in full_groups]
        + [g * S for _, g, S, _ in rag_groups]
        + [1]
    )
    max_g = max(
        [g for _, _, _, g, _, _ in full_groups]
        + [g for _, g, _, _ in rag_groups]
        + [1]
    )
    return dict(
        ks=ks, cls=cls, full_groups=full_groups, rag_groups=rag_groups,
        nfullwin=fullwin, RW=RW, nwin=fullwin + RW, s_elems=max(base, 1),
        kwin=kwin, karr=karr, max_gs=max_gs, max_g=max_g,
        dve_cls=dve_cls, dve_rag=dve_rag,
    )


def _build_nc(plan, nb):
    ks = plan["ks"]
    ncols = plan["nwin"] * 128
    nrw = plan["RW"]
    jW1un = len(ks)       # unscaled BD(W1.T) for ragged sums
    jW2 = len(ks) + 1     # BD(W2.T) for max projection

    nc = bacc.Bacc(None)
    slots_t = nc.dram_tensor("slots", [plan["s_elems"]], BF16, kind="ExternalInput")
    wts_t = nc.dram_tensor("wts", [nb * 128, 128], BF16, kind="ExternalInput")
    b_t = nc.dram_tensor("b", [128, 1], FP32, kind="ExternalInput")
    if nrw:
        rt_t = nc.dram_tensor("rt", [128, nrw * 128], BF16, kind="ExternalInput")
    out_t = nc.dram_tensor("out", [128, ncols], BF16, kind="ExternalOutput")

    INF = plan["max_gs"] * 128  # in-tile free elems
    MAXG = plan["max_g"]

    with tile.TileContext(nc) as tc:
        with (
            tc.tile_pool(name="const", bufs=1) as constp,
            tc.tile_pool(name="inp", bufs=4) as inp,
            tc.tile_pool(name="treep", bufs=2) as treep,
            tc.tile_pool(name="tmpp", bufs=2) as tmpp,
            tc.tile_pool(name="outp", bufs=2) as outp,
            tc.tile_pool(name="psA", bufs=6, space="PSUM") as psA,
            tc.tile_pool(name="psB", bufs=2, space="PSUM") as psB,
        ):
            wt_sb = constp.tile([128, nb * 128], BF16)
            nc.sync.dma_start(
                wt_sb[:].rearrange("p (j m) -> p j m", m=128),
                wts_t[:, :].rearrange("(j p) m -> p j m", p=128),
            )
            b_sb = constp.tile([128, 1], FP32)
            nc.sync.dma_start(b_sb[:], b_t[:])
            if nrw:
                rt_sb = constp.tile([128, nrw * 128], BF16)
                nc.sync.dma_start(rt_sb[:], rt_t[:, :])

            def lhs(j):
                return wt_sb[:, j * 128 : (j + 1) * 128]

            def tree(v, g, n, op, tag):
                """DVE pairwise reduction tree over the slot axis of a 4D
                view [p, gg, s, d]; one tensor_tensor per level (2x_1p in
                fp16). Returns [p, gg, d]."""
                cur = v
                carries = []
                lvl = 0
                while n > 1:
                    h = n // 2
                    if n % 2:
                        carries.append(cur[:, :, n - 1, :])
                    lvl += 1
                    tnew = treep.tile(
                        [128, max(INF // (2 ** lvl), 128)], BF16,
                        tag=f"{tag}{lvl}",
                    )
                    ov = tnew[:, : g * h * 128].rearrange(
                        "p (gg s d) -> p gg s d", gg=g, s=h
                    )
                    nc.vector.tensor_tensor(
                        out=ov, in0=cur[:, :, 0 : 2 * h : 2, :],
                        in1=cur[:, :, 1 : 2 * h : 2, :], op=op,
                    )
                    cur = ov
                    n = h
                m3 = cur[:, :, 0, :]  # [p, gg, d]
                for cv in carries:
                    tcar = treep.tile([128, MAXG * 128], BF16, tag=f"{tag}c")
                    c3 = tcar[:, : g * 128].rearrange("p (gg d) -> p gg d", gg=g)
                    nc.vector.tensor_tensor(out=c3, in0=m3, in1=cv, op=op)
                    m3 = c3
                return m3

            def do_group(S, g, base, col0, jsum, ragged, dve_sum, rtc0=0):
                F = g * S * 128
                in_t = inp.tile([128, INF], BF16, tag="in")
                nc.sync.dma_start(
                    in_t[:, :F].rearrange("p (gg f) -> p gg f", gg=g),
                    slots_t[base : base + 128 * F].rearrange(
                        "(gg p f) -> p gg f", gg=g, p=128
                    ),
                )
                v = in_t[:, :F].rearrange("p (gg s d) -> p gg s d", gg=g, s=S)

                # max tree first: it gates the PE's W2 projection, so it
                # should land before the sum tree on the in-order DVE.
                S_tree = S - 1 if ragged else S
                m3 = tree(v, g, S_tree, mybir.AluOpType.max, "m")
                # segment sum: either a DVE add-tree (then one projection
                # matmul per chunk), or slot-fused matmuls on the PE (stride-0
                # out AP accumulates 4 slots per instruction).
                sum3 = tree(v, g, S, mybir.AluOpType.add, "s") if dve_sum else None

                chunks = []
                for c0 in range(0, g, 4):
                    cw = min(4, g - c0)
                    ps = psA.tile([128, 512], FP32, space="PSUM", tag="ps")
                    if dve_sum:
                        if not ragged:
                            # W2*max first: the max tree lands on DVE before
                            # the sum tree, so this unblocks the PE earlier.
                            nc.tensor.matmul(
                                ps[:, : cw * 128],
                                lhsT=lhs(jW2),
                                rhs=m3[:, c0 : c0 + cw, :],
                                start=True,
                                stop=False,
                            )
                        nc.tensor.matmul(
                            ps[:, : cw * 128],
                            lhsT=lhs(jsum),
                            rhs=sum3[:, c0 : c0 + cw, :],
                            start=bool(ragged),
                            stop=True,
                        )
                    else:
                        for ci in range(cw):
                            # ISA: matmul rhs <= 512 elems/partition, APs 3D
                            # max. start=True only on the bank's FIRST matmul:
                            # start resets the bank-wide has-written bits, so
                            # a later start would make subsequent writes
                            # overwrite earlier windows' accumulated sums.
                            for s0 in range(0, S, 4):
                                sr = min(4, S - s0)
                                out_b = (
                                    ps[:, ci * 128 : (ci + 1) * 128]
                                    .rearrange("p (one d) -> p one d", one=1)
                                    .broadcast_to([128, sr, 128])
                                )
                                nc.tensor.matmul(
                                    out_b,
                                    lhsT=lhs(jsum),
                                    rhs=v[:, c0 + ci, s0 : s0 + sr, :],
                                    start=(ci == 0 and s0 == 0),
                                    stop=bool(ragged)
                                    and ci == cw - 1
                                    and s0 + sr >= S,
                                )
                    chunks.append((c0, cw, ps))

                # ---- PE: max projection; ACT/DVE: bias + writeout -------
                out_sb = outp.tile([128, MAXG * 128], BF16, tag="out")
                for c0, cw, ps in chunks:
                    rhs_m = m3[:, c0 : c0 + cw, :]
                    oc = out_sb[:, c0 * 128 : (c0 + cw) * 128]
                    if not ragged:
                        if not dve_sum:  # dve_sum emitted W2 in the loop above
                            nc.tensor.matmul(
                                ps[:, : cw * 128], lhsT=lhs(jW2), rhs=rhs_m,
                                start=False, stop=True,
                            )
                        nc.scalar.activation(
                            oc, ps[:, : cw * 128],
                            mybir.ActivationFunctionType.Identity,
                            bias=b_sb[:, :1],
                        )
                    else:
                        psb = psB.tile([128, 512], FP32, space="PSUM", tag="psb")
                        nc.tensor.matmul(
                            psb[:, : cw * 128], lhsT=lhs(jW2), rhs=rhs_m,
                            start=True, stop=True,
                        )
                        tmp = tmpp.tile([128, MAXG * 128], BF16, tag="tmp")
                        tc_ = tmp[:, : cw * 128]
                        nc.vector.tensor_mul(
                            tc_, ps[:, : cw * 128],
                            rt_sb[:, rtc0 + c0 * 128 : rtc0 + (c0 + cw) * 128],
                        )
                        nc.vector.scalar_tensor_tensor(
                            out=oc, in0=psb[:, : cw * 128], scalar=b_sb[:, :1],
                            in1=tc_, op0=mybir.AluOpType.add,
                            op1=mybir.AluOpType.add,
                        )
                nc.scalar.dma_start(
                    out_t[:, col0 : col0 + g * 128], out_sb[:, : g * 128]
                )

            # Interleave ragged groups among the full groups (never last):
            # ragged groups have longer per-group dependency chains, so
            # placing one at the program tail adds un-overlapped latency.
            jobs = [
                (
                    "full",
                    (S, g, base, win0 * 128, jcls),
                    k in plan["dve_cls"],
                )
                for k, jcls, win0, g, S, base in plan["full_groups"]
            ]
            rjobs = [
                (
                    "rag",
                    (
                        S, g, base, (plan["nfullwin"] + win0_r) * 128,
                        jW1un, win0_r * 128,
                    ),
                    gi in plan["dve_rag"],
                )
                for gi, (win0_r, g, S, base) in enumerate(plan["rag_groups"])
            ]
            ordered = jobs + rjobs
            if os.environ.get("KERNEL_INTERLEAVE", "0") == "1" and jobs and rjobs:
                nf = len(jobs)
                step = max(nf // (len(rjobs) + 1), 1)
                ordered = []
                ri = 0
                for i, jb in enumerate(jobs):
                    ordered.append(jb)
                    if ri < len(rjobs) and (i + 1) % step == 0 and i < nf - 2:
                        ordered.append(rjobs[ri])
                        ri += 1
                ins = max(len(ordered) - 2, 0)
                ordered[ins:ins] = rjobs[ri:]
            for kind, args, dvs in ordered:
                if kind == "full":
                    S, g, base, col0, jcls = args
                    do_group(S, g, base, col0, jcls, ragged=False, dve_sum=dvs)
                else:
                    S, g, base, col0, jsum, rtc0 = args
                    do_group(
                        S, g, base, col0, jsum, ragged=True, dve_sum=dvs,
                        rtc0=rtc0,
                    )
    nc.finalize()
    return nc


def kernel(pulse_embeddings, pulse_to_dom_idx, num_doms, proj_w, proj_b):
    global last_exec_ns

    E = np.asarray(pulse_embeddings, dtype=np.float32)
    idx = np.asarray(pulse_to_dom_idx).astype(np.int64)
    nd = int(num_doms)
    W = np.asarray(proj_w, dtype=np.float32)   # (D, 2D)
    b = np.asarray(proj_b, dtype=np.float32)   # (D,)

    counts = np.bincount(idx, minlength=nd).astype(np.int64)
    plan = _plan(counts)
    ks = plan["ks"]
    cls = plan["cls"]
    nb = len(ks) + 2

    # ---- dom assignment (vectorized, shared structure) -------------------
    order = np.argsort(counts, kind="stable")
    n0 = int((counts == 0).sum())
    dom_core = np.full(nd, -1, np.int32)
    dom_win = np.zeros(nd, np.int64)
    dom_half = np.zeros(nd, np.int32)
    dom_dp = np.zeros(nd, np.int32)
    nfullwin = plan["nfullwin"]

    off = n0
    for k in ks:
        n, m, fw, r, rag_off_k, fullwin0 = cls[k]
        doms = order[off : off + n]
        off += n
        core = (np.arange(n) % NCORES).astype(np.int32)
        pos = (np.arange(n) // NCORES).astype(np.int64)
        isfull = pos < fw * 256
        rpos = rag_off_k + (pos - fw * 256)
        q = np.where(isfull, pos % 256, rpos % 256)
        win = np.where(isfull, fullwin0 + pos // 256, nfullwin + rpos // 256)
        dom_core[doms] = core
        dom_win[doms] = win
        dom_half[doms] = (q // 128).astype(np.int32)
        dom_dp[doms] = (q % 128).astype(np.int32)

    # ---- pulse permutation: sort by (core, window, half, dp) -------------
    dkey = (
        dom_core.astype(np.int64) * (1 << 40)
        + dom_win * 256
        + dom_half * 128
        + dom_dp
    )
    pkey = dkey[idx]
    perm = np.argsort(pkey, kind="stable")
    core_pulse_counts = np.bincount(dom_core[idx], minlength=NCORES)
    core_splits = np.concatenate([[0], np.cumsum(core_pulse_counts)])

    # ---- host-side slot buffer packing -----------------------------------
    E16 = E.astype(BF)
    s_elems = plan["s_elems"]
    RW = plan["RW"]

    # ragged group lookup per ragged window
    if RW:
        grp_of_rw = np.zeros(RW, np.int32)
        for gi, (win0_r, g, S, base) in enumerate(plan["rag_groups"]):
            grp_of_rw[win0_r : win0_r + g] = gi

    # full groups of each class, in order
    fg_of_k = {}
    for fg in plan["full_groups"]:
        fg_of_k.setdefault(fg[0], []).append(fg)

    e_idx = np.arange(D)
    bufs = []
    for c in range(NCORES):
        buf = np.zeros(s_elems, BF)
        pc = perm[core_splits[c] : core_splits[c + 1]]
        p = 0
        # full regions, class ascending
        for k in ks:
            n, m, fw, r, rag_off_k, fullwin0 = cls[k]
            nreal_c = n // NCORES + (1 if c < n % NCORES else 0)
            n_full = min(nreal_c, fw * 256)
            if fw == 0:
                continue
            if n_full == 0:
                continue
            rows = pc[p : p + n_full * k].reshape(n_full, k)
            p += n_full * k
            A = E16[rows]  # (n_full, k, 64)
            for (_, _, win0, g, S, base) in fg_of_k[k]:
                lo = (win0 - fullwin0) * 256
                hi = min(n_full, lo + g * 256)
                if hi <= lo:
                    continue
                T = buf[base : base + g * 128 * k * 128].reshape(g, 128, k, 128)
                nw_full = (hi - lo) // 256
                if nw_full:
                    B = (
                        A[lo : lo + nw_full * 256]
                        .reshape(nw_full, 2, 128, k, D)
                        .transpose(0, 1, 4, 3, 2)
                        .reshape(nw_full, 128, k, 128)
                    )
                    T[:nw_full] = B
                rem = (hi - lo) - nw_full * 256
                if rem:
                    Ar = A[lo + nw_full * 256 : hi]  # (rem, k, 64)
                    j = np.arange(rem)
                    h = (j // 128).astype(np.int32)
                    dp = (j % 128).astype(np.int32)
                    T[
                        nw_full,
                        (64 * h)[:, None, None] + e_idx[None, None, :],
                        np.arange(k)[None, :, None],
                        dp[:, None, None],
                    ] = Ar
        # ragged region, class ascending
        for k in ks:
            n, m, fw, r, rag_off_k, fullwin0 = cls[k]
            if r == 0:
                continue
            nreal_c = n // NCORES + (1 if c < n % NCORES else 0)
            n_full = min(nreal_c, fw * 256)
            n_rag = nreal_c - n_full
            if n_rag == 0:
                continue
            rows = pc[p : p + n_rag * k].reshape(n_rag, k)
            p += n_rag * k
            A = E16[rows]  # (n_rag, k, 64)
            rpos = rag_off_k + np.arange(n_rag)
            rw = rpos // 256
            grp = grp_of_rw[rw]
            bnds = np.concatenate(
                [[0], np.nonzero(np.diff(grp))[0] + 1, [n_rag]]
            )
            for bi in range(len(bnds) - 1):
                j0, j1 = int(bnds[bi]), int(bnds[bi + 1])
                gi = int(grp[j0])
                win0_r, g, S, base = plan["rag_groups"][gi]
                Ac = A[j0:j1]
                nn = j1 - j0
                Bf = np.empty((nn, S, D), BF)
                Bf[:, :k] = Ac
                padcnt = (S - 1) - k
                if padcnt > 0:
                    Bf[:, k : S - 1] = Ac[:, :1]
                Bf[:, S - 1] = (
                    -float(padcnt) * Ac[:, 0].astype(np.float32)
                ).astype(BF)
                T = buf[base : base + g * 128 * S * 128].reshape(g, 128, S, 128)
                rw_l = (rw[j0:j1] - win0_r).astype(np.int64)
                h = ((rpos[j0:j1] % 256) // 128).astype(np.int64)
                dp = (rpos[j0:j1] % 128).astype(np.int64)
                T[
                    rw_l[:, None, None],
                    (64 * h)[:, None, None] + e_idx[None, None, :],
                    np.arange(S)[None, :, None],
                    dp[:, None, None],
                ] = Bf
        bufs.append(buf)

    # ---- weights / bias / recip table ------------------------------------
    W1t = np.ascontiguousarray(W[:, :D].T)   # (64, 64) = W1.T
    W2t = np.ascontiguousarray(W[:, D:].T)
    wts = np.zeros((nb * 128, 128), np.float32)
    for j, k in enumerate(ks):
        blk = W1t / np.float32(k)
        wts[j * 128 : j * 128 + 64, :64] = blk
        wts[j * 128 + 64 : (j + 1) * 128, 64:] = blk
    j = len(ks)
    wts[j * 128 : j * 128 + 64, :64] = W1t
    wts[j * 128 + 64 : (j + 1) * 128, 64:] = W1t
    j = len(ks) + 1
    wts[j * 128 : j * 128 + 64, :64] = W2t
    wts[j * 128 + 64 : (j + 1) * 128, 64:] = W2t
    wts16 = wts.astype(BF)
    b2 = np.concatenate([b, b]).reshape(128, 1).astype(np.float32)

    rt = None
    if RW:
        karr = plan["karr"].astype(np.float64)
        recip = np.where(karr > 0, 1.0 / np.maximum(karr, 1), 0.0).astype(BF)
        rec = recip.reshape(RW, 2, 128)
        rt = np.zeros((128, RW * 128), BF)
        rt4 = rt.reshape(128, RW, 128)
        rt4[0:64] = rec[:, 0, :][None]
        rt4[64:128] = rec[:, 1, :][None]

    # ---- optional numpy emulation of the device program (KERNEL_SIM=1) ----
    if os.environ.get("KERNEL_SIM", "0") == "1":
        outs = np.stack(
            [_simulate(plan, bufs[c], wts, b2, rt, len(ks)) for c in range(NCORES)]
        )
        last_exec_ns = None
        return _unpermute(
            outs, plan, dom_core, dom_win, dom_half, dom_dp, nd, n0, b
        )

    # ---- device -----------------------------------------------------------
    if os.environ.get("KERNEL_LDW_OPT", "1") == "1":
        try:
            from concourse.compiler_utils import (
                get_compiler_flags,
                set_compiler_flags,
            )

            flags = [
                f.replace("--enable-ldw-opt=false", "--enable-ldw-opt=true")
                for f in get_compiler_flags()
            ]
            set_compiler_flags(flags)
        except Exception:
            pass
    nc = _build_nc(plan, nb)
    in_maps = []
    for c in range(NCORES):
        m = {"slots": bufs[c], "wts": wts16, "b": b2}
        if RW:
            m["rt"] = rt
        in_maps.append(m)
    trace = os.environ.get("KERNEL_TRACE", "0") == "1"
    kw_ = {}
    if trace and _install_trace_hook():
        import tempfile

        kw_ = dict(trace=True, tmpdir=tempfile.mkdtemp(prefix="kernel_trace_"))
    res = run_bass_kernel_spmd(nc, in_maps, core_ids=list(range(NCORES)), **kw_)
    last_exec_ns = res.exec_time_ns

    # ---- host-side unpermute ----------------------------------------------
    outs = np.stack(
        [np.asarray(res.results[c]["out"], dtype=np.float32) for c in range(NCORES)]
    )  # (8,128,ncols)
    return _unpermute(outs, plan, dom_core, dom_win, dom_half, dom_dp, nd, n0, b)


def _unpermute(outs, plan, dom_core, dom_win, dom_half, dom_dp, nd, n0, b):
    ncols = plan["nwin"] * 128
    outs_t = outs.transpose(0, 2, 1).reshape(NCORES, ncols, 2, D)
    full = np.empty((nd, D), np.float32)
    real = dom_core >= 0
    col = dom_win * 128 + dom_dp
    full[real] = outs_t[dom_core[real], col[real], dom_half[real]]
    if n0:
        full[~real] = b
    return full


def _simulate(plan, buf, wts, b2, rt, ncls):
    """Numpy emulation of the device program for one core (fp32 math)."""
    ncols = plan["nwin"] * 128
    out = np.zeros((128, ncols), np.float32)
    jW1un, jW2 = ncls, ncls + 1

    def blkT(j):
        return wts[j * 128 : (j + 1) * 128].astype(np.float32).T  # (128out,128in)

    for k, jcls, win0, g, S, base in plan["full_groups"]:
        T = buf[base : base + g * 128 * S * 128].reshape(g, 128, S, 128)
        Tf = T.astype(np.float32)
        ssum = Tf.sum(axis=2)          # (g, 128, 128)
        smax = Tf.max(axis=2)          # bf16-exact max
        for w in range(g):
            ps = blkT(jcls) @ ssum[w] + blkT(jW2) @ smax[w]
            out[:, (win0 + w) * 128 : (win0 + w + 1) * 128] = ps + b2
    for win0_r, g, S, base in plan["rag_groups"]:
        T = buf[base : base + g * 128 * S * 128].reshape(g, 128, S, 128)
        Tf = T.astype(np.float32)
        ssum = Tf.sum(axis=2)
        smax = Tf[:, :, : S - 1, :].max(axis=2)
        for w in range(g):
            rw = win0_r + w
            mean = (blkT(jW1un) @ ssum[w]) * rt[:, rw * 128 : (rw + 1) * 128]
            ps = mean + blkT(jW2) @ smax[w]
            out[:, (plan["nfullwin"] + rw) * 128 : (plan["nfullwin"] + rw + 1) * 128] = (
                ps + b2
            )
    return out


# revision 28
# speedup vs baseline: 1.1216x; 1.0178x over previous
"""DOM pooling (segment mean+max over pulses, then linear projection) on 8 trn2 cores.

v2 strategy (vs baseline's dom-in-partition two-pass tensor_reduce, which is
DVE-bound at 1x mode):
  Layout: embed-in-partition. A window = 128 "dompair" columns; partition rows
  0-63 hold the 64 embed dims of the top dom, rows 64-127 the bottom dom.
  A dom's k pulse slots are k consecutive 128-column blocks. Data stored bf16
  (tolerance 2e-2; bf16 adds ~4e-3), halving HBM traffic.

  Per window group on device:
    - segment SUM + mean-scaling + projection fused on the TensorE: one
      accumulating matmul per slot into PSUM with per-class block-diag
      weights block_diag((W1/k).T, (W1/k).T).
    - segment MAX on DVE as a pairwise tensor_max tree (2x_1p mode in bf16,
      one instruction per level via strided views) instead of 1x tensor_reduce.
    - max projection: one more accumulating matmul with block_diag(W2.T, W2.T)
      into the same PSUM -> full concat projection accumulated in place.
    - ACT adds bias during PSUM->SBUF copy; DMA writes [128, cols] f32 out.

  Doms are bucketed by exact pulse count k (class) and dealt round-robin over
  the 8 cores (identical structure per core -> one SPMD NEFF). Full windows
  hold one class (k uniform, 1/k folded into weights). Per-class leftovers are
  packed into shared "ragged" windows (sorted by k, slots padded to the group
  max with dup-of-slot-0 which is max-neutral, plus one host-built correction
  slot -padcnt*x0 so the PE sum stays exact); the per-dom 1/k is applied via
  an elementwise recip table multiply on DVE, then max-proj + bias added with
  one scalar_tensor_tensor.
"""
import os
import sys

import numpy as np

for _p in ("/opt/trn_rl_repo",):
    if _p not in sys.path:
        sys.path.append(_p)

import ml_dtypes
from concourse import bacc
import concourse.mybir as mybir
import concourse.tile as tile
from concourse.bass_utils import run_bass_kernel_spmd

NCORES = 8
D = 64
FP32 = mybir.dt.float32
BF16 = mybir.dt.float16
BF = np.float16

GS_BUDGET = 112   # max g*S per group (SBUF in-tile = [128, GS_BUDGET*128] bf16)
GMAX = 14        # max windows per group

last_exec_ns = None  # set when KERNEL_TRACE=1


def _install_trace_hook():
    """Best-effort: synthesize antenv.axon_hooks + NTFF hook so
    run_bass_kernel_spmd(trace=True) works under axon."""
    import types

    try:
        if "antenv.axon_hooks" in sys.modules:
            return True
        import antenv

        mod = types.ModuleType("antenv.axon_hooks")
        mod._hook = None
        mod.set_axon_ntff_profile_hook = lambda h: setattr(mod, "_hook", h)
        mod.get_axon_ntff_profile_hook = lambda: mod._hook
        sys.modules["antenv.axon_hooks"] = mod
        antenv.axon_hooks = mod
        from trn_agent_boot.trn_boot import _ntff_profile_via_ctypes

        hook = _ntff_profile_via_ctypes("/opt/axon/libaxon_pjrt.so")
        if hook is not None:
            mod._hook = hook
        return hook is not None
    except Exception:
        return False


def _plan(counts):
    """Shared (all-core) window/group plan derived from global counts.

    Returns dict with:
      ks: ascending class pulse-counts
      cls: {k: (n_k, m_k, fw_k, r_k, rag_off_k, fullwin0_k)}
      full_groups: [(k, jcls, win0, g, S, base)]
      rag_groups:  [(win0_r, g, S, base)]      win0_r is ragged-window index
      nfullwin, RW, nwin, s_elems, kwin (per ragged window), karr (rag layout)
    """
    nd = counts.shape[0]
    kmax = int(counts.max()) if nd else 0
    n_k = np.bincount(counts, minlength=kmax + 1)
    ks = [k for k in range(1, kmax + 1) if n_k[k] > 0]

    cls = {}
    rag_off = 0
    fullwin = 0
    for k in ks:
        n = int(n_k[k])
        m = -(-n // NCORES)
        fw = m // 256
        r = m % 256
        cls[k] = [n, m, fw, r, rag_off, fullwin]
        rag_off += r
        fullwin += fw
    R = rag_off
    RW = -(-R // 256) if R else 0

    # ragged layout: class k occupies rpos [rag_off_k, rag_off_k + r_k)
    karr = np.zeros(RW * 256, np.int32)
    for k in ks:
        _, _, _, r, ro, _ = cls[k]
        if r:
            karr[ro : ro + r] = k
    kwin = karr.reshape(RW, 256).max(axis=1) if RW else np.zeros(0, np.int32)

    # full groups
    full_groups = []
    base = 0
    for jcls, k in enumerate(ks):
        _, _, fw, _, _, fullwin0 = cls[k]
        if fw == 0:
            continue
        g_cap = min(GMAX, max(1, GS_BUDGET // k))
        w = 0
        while w < fw:
            g = min(g_cap, fw - w)
            full_groups.append((k, jcls, fullwin0 + w, g, k, base))
            base += g * 128 * k * 128
            w += g

    # ragged groups (greedy, S = max kw in group + 1; cap pad growth so a
    # group never stores more than ~1.15x the per-window-exact slot count)
    rag_groups = []
    rw = 0
    while rw < RW:
        g = 1
        S = int(kwin[rw]) + 1
        exact = S
        while rw + g < RW:
            S2 = max(S, int(kwin[rw + g]) + 1)
            exact2 = exact + int(kwin[rw + g]) + 1
            if (
                (g + 1) * S2 > GS_BUDGET
                or g + 1 > GMAX
                or (g + 1) * S2 > float(os.environ.get("KERNEL_RAG_PAD_CAP", "9.9")) * exact2
            ):
                break
            S = S2
            exact = exact2
            g += 1
        rag_groups.append((rw, g, S, base))
        base += g * 128 * S * 128
        rw += g

    # Select which classes/groups compute their segment-sum on the DVE
    # (add-tree) instead of the PE, to balance the two engines. Units with
    # the worst PE efficiency (ceil(S/4)*4/S) are offloaded first, up to
    # DVE_SUM_FRAC of the total slot elements.
    frac = float(os.environ.get("KERNEL_DVE_FRAC", "0"))
    units = []
    for k in ks:
        if cls[k][2]:
            el = sum(g * S * 128 * 128 for kk, _, _, g, S, _ in full_groups if kk == k)
            units.append((-(-k // 4) * 4 / k, el, ("cls", k)))
    for gi, (_, g, S, _) in enumerate(rag_groups):
        el = g * S * 128 * 128
        units.append((-(-S // 4) * 4 / S, el, ("rag", gi)))
    units.sort(key=lambda u: -u[0])
    target = frac * max(base, 1)
    acc = 0
    dve_cls = set()
    dve_rag = set()
    for _, el, (kind, ident) in units:
        if acc >= target:
            break
        acc += el
        (dve_cls if kind == "cls" else dve_rag).add(ident)

    max_gs = max(
        [g * S for _, _, _, g, S, _ in full_groups]
        + [g * S for _, g, S, _ in rag_groups]
        + [1]
    )
    max_g = max(
        [g for _, _, _, g, _, _ in full_groups]
        + [g for _, g, _, _ in rag_groups]
        + [1]
    )
    return dict(
        ks=ks, cls=cls, full_groups=full_groups, rag_groups=rag_groups,
        nfullwin=fullwin, RW=RW, nwin=fullwin + RW, s_elems=max(base, 1),
        kwin=kwin, karr=karr, max_gs=max_gs, max_g=max_g,
        dve_cls=dve_cls, dve_rag=dve_rag,
    )


def _build_nc(plan, nb):
    ks = plan["ks"]
    ncols = plan["nwin"] * 128
    nrw = plan["RW"]
    jW1un = len(ks)       # unscaled BD(W1.T) for ragged sums
    jW2 = len(ks) + 1     # BD(W2.T) for max projection

    nc = bacc.Bacc(None)
    slots_t = nc.dram_tensor("slots", [plan["s_elems"]], BF16, kind="ExternalInput")
    wts_t = nc.dram_tensor("wts", [nb * 128, 128], BF16, kind="ExternalInput")
    b_t = nc.dram_tensor("b", [128, 1], FP32, kind="ExternalInput")
    if nrw:
        rt_t = nc.dram_tensor("rt", [128, nrw * 128], BF16, kind="ExternalInput")
    out_t = nc.dram_tensor("out", [128, ncols], BF16, kind="ExternalOutput")

    INF = plan["max_gs"] * 128  # in-tile free elems
    MAXG = plan["max_g"]

    with tile.TileContext(nc) as tc:
        with (
            tc.tile_pool(name="const", bufs=1) as constp,
            tc.tile_pool(name="inp", bufs=4) as inp,
            tc.tile_pool(name="treep", bufs=2) as treep,
            tc.tile_pool(name="tmpp", bufs=2) as tmpp,
            tc.tile_pool(name="outp", bufs=2) as outp,
            tc.tile_pool(name="psA", bufs=6, space="PSUM") as psA,
            tc.tile_pool(name="psB", bufs=2, space="PSUM") as psB,
        ):
            wt_sb = constp.tile([128, nb * 128], BF16)
            nc.sync.dma_start(
                wt_sb[:].rearrange("p (j m) -> p j m", m=128),
                wts_t[:, :].rearrange("(j p) m -> p j m", p=128),
            )
            b_sb = constp.tile([128, 1], FP32)
            nc.sync.dma_start(b_sb[:], b_t[:])
            if nrw:
                rt_sb = constp.tile([128, nrw * 128], BF16)
                nc.sync.dma_start(rt_sb[:], rt_t[:, :])

            def lhs(j):
                return wt_sb[:, j * 128 : (j + 1) * 128]

            def tree(v, g, n, op, tag):
                """DVE pairwise reduction tree over the slot axis of a 4D
                view [p, gg, s, d]; one tensor_tensor per level (2x_1p in
                fp16). Returns [p, gg, d]."""
                cur = v
                carries = []
                lvl = 0
                while n > 1:
                    h = n // 2
                    if n % 2:
                        carries.append(cur[:, :, n - 1, :])
                    lvl += 1
                    tnew = treep.tile(
                        [128, max(INF // (2 ** lvl), 128)], BF16,
                        tag=f"{tag}{lvl}",
                    )
                    ov = tnew[:, : g * h * 128].rearrange(
                        "p (gg s d) -> p gg s d", gg=g, s=h
                    )
                    nc.vector.tensor_tensor(
                        out=ov, in0=cur[:, :, 0 : 2 * h : 2, :],
                        in1=cur[:, :, 1 : 2 * h : 2, :], op=op,
                    )
                    cur = ov
                    n = h
                m3 = cur[:, :, 0, :]  # [p, gg, d]
                for cv in carries:
                    tcar = treep.tile([128, MAXG * 128], BF16, tag=f"{tag}c")
                    c3 = tcar[:, : g * 128].rearrange("p (gg d) -> p gg d", gg=g)
                    nc.vector.tensor_tensor(out=c3, in0=m3, in1=cv, op=op)
                    m3 = c3
                return m3

            def do_group(S, g, base, col0, jsum, ragged, dve_sum, rtc0=0):
                F = g * S * 128
                in_t = inp.tile([128, INF], BF16, tag="in")
                nc.sync.dma_start(
                    in_t[:, :F].rearrange("p (gg f) -> p gg f", gg=g),
                    slots_t[base : base + 128 * F].rearrange(
                        "(gg p f) -> p gg f", gg=g, p=128
                    ),
                )
                v = in_t[:, :F].rearrange("p (gg s d) -> p gg s d", gg=g, s=S)

                # max tree first: it gates the PE's W2 projection, so it
                # should land before the sum tree on the in-order DVE.
                S_tree = S - 1 if ragged else S
                m3 = tree(v, g, S_tree, mybir.AluOpType.max, "m")
                # segment sum: either a DVE add-tree (then one projection
                # matmul per chunk), or slot-fused matmuls on the PE (stride-0
                # out AP accumulates 4 slots per instruction).
                sum3 = tree(v, g, S, mybir.AluOpType.add, "s") if dve_sum else None

                chunks = []
                for c0 in range(0, g, 4):
                    cw = min(4, g - c0)
                    ps = psA.tile([128, 512], FP32, space="PSUM", tag="ps")
                    if dve_sum:
                        if not ragged:
                            # W2*max first: the max tree lands on DVE before
                            # the sum tree, so this unblocks the PE earlier.
                            nc.tensor.matmul(
                                ps[:, : cw * 128],
                                lhsT=lhs(jW2),
                                rhs=m3[:, c0 : c0 + cw, :],
                                start=True,
                                stop=False,
                            )
                        nc.tensor.matmul(
                            ps[:, : cw * 128],
                            lhsT=lhs(jsum),
                            rhs=sum3[:, c0 : c0 + cw, :],
                            start=bool(ragged),
                            stop=True,
                        )
                    else:
                        for ci in range(cw):
                            # ISA: matmul rhs <= 512 elems/partition, APs 3D
                            # max. start=True only on the bank's FIRST matmul:
                            # start resets the bank-wide has-written bits, so
                            # a later start would make subsequent writes
                            # overwrite earlier windows' accumulated sums.
                            for s0 in range(0, S, 4):
                                sr = min(4, S - s0)
                                out_b = (
                                    ps[:, ci * 128 : (ci + 1) * 128]
                                    .rearrange("p (one d) -> p one d", one=1)
                                    .broadcast_to([128, sr, 128])
                                )
                                nc.tensor.matmul(
                                    out_b,
                                    lhsT=lhs(jsum),
                                    rhs=v[:, c0 + ci, s0 : s0 + sr, :],
                                    start=(ci == 0 and s0 == 0),
                                    stop=bool(ragged)
                                    and ci == cw - 1
                                    and s0 + sr >= S,
                                )
                    chunks.append((c0, cw, ps))

                # ---- PE: max projection; ACT/DVE: bias + writeout -------
                out_sb = outp.tile([128, MAXG * 128], BF16, tag="out")
                for c0, cw, ps in chunks:
                    rhs_m = m3[:, c0 : c0 + cw, :]
                    oc = out_sb[:, c0 * 128 : (c0 + cw) * 128]
                    if not ragged:
                        if not dve_sum:  # dve_sum emitted W2 in the loop above
                            nc.tensor.matmul(
                                ps[:, : cw * 128], lhsT=lhs(jW2), rhs=rhs_m,
                                start=False, stop=True,
                            )
                        nc.scalar.activation(
                            oc, ps[:, : cw * 128],
                            mybir.ActivationFunctionType.Identity,
                            bias=b_sb[:, :1],
                        )
                    else:
                        psb = psB.tile([128, 512], FP32, space="PSUM", tag="psb")
                        nc.tensor.matmul(
                            psb[:, : cw * 128], lhsT=lhs(jW2), rhs=rhs_m,
                            start=True, stop=True,
                        )
                        tmp = tmpp.tile([128, MAXG * 128], BF16, tag="tmp")
                        tc_ = tmp[:, : cw * 128]
                        nc.vector.tensor_mul(
                            tc_, ps[:, : cw * 128],
                            rt_sb[:, rtc0 + c0 * 128 : rtc0 + (c0 + cw) * 128],
                        )
                        nc.vector.scalar_tensor_tensor(
                            out=oc, in0=psb[:, : cw * 128], scalar=b_sb[:, :1],
                            in1=tc_, op0=mybir.AluOpType.add,
                            op1=mybir.AluOpType.add,
                        )
                nc.scalar.dma_start(
                    out_t[:, col0 : col0 + g * 128], out_sb[:, : g * 128]
                )

            # Interleave ragged groups among the full groups (never last):
            # ragged groups have longer per-group dependency chains, so
            # placing one at the program tail adds un-overlapped latency.
            jobs = [
                (
                    "full",
                    (S, g, base, win0 * 128, jcls),
                    k in plan["dve_cls"],
                )
                for k, jcls, win0, g, S, base in plan["full_groups"]
            ]
            rjobs = [
                (
                    "rag",
                    (
                        S, g, base, (plan["nfullwin"] + win0_r) * 128,
                        jW1un, win0_r * 128,
                    ),
                    gi in plan["dve_rag"],
                )
                for gi, (win0_r, g, S, base) in enumerate(plan["rag_groups"])
            ]
            ordered = jobs + rjobs
            if os.environ.get("KERNEL_INTERLEAVE", "0") == "1" and jobs and rjobs:
                nf = len(jobs)
                step = max(nf // (len(rjobs) + 1), 1)
                ordered = []
                ri = 0
                for i, jb in enumerate(jobs):
                    ordered.append(jb)
                    if ri < len(rjobs) and (i + 1) % step == 0 and i < nf - 2:
                        ordered.append(rjobs[ri])
                        ri += 1
                ins = max(len(ordered) - 2, 0)
                ordered[ins:ins] = rjobs[ri:]
            for kind, args, dvs in ordered:
                if kind == "full":
                    S, g, base, col0, jcls = args
                    do_group(S, g, base, col0, jcls, ragged=False, dve_sum=dvs)
                else:
                    S, g, base, col0, jsum, rtc0 = args
                    do_group(
                        S, g, base, col0, jsum, ragged=True, dve_sum=dvs,
                        rtc0=rtc0,
                    )
    nc.finalize()
    return nc


def kernel(pulse_embeddings, pulse_to_dom_idx, num_doms, proj_w, proj_b):
    global last_exec_ns

    E = np.asarray(pulse_embeddings, dtype=np.float32)
    idx = np.asarray(pulse_to_dom_idx).astype(np.int64)
    nd = int(num_doms)
    W = np.asarray(proj_w, dtype=np.float32)   # (D, 2D)
    b = np.asarray(proj_b, dtype=np.float32)   # (D,)

    counts = np.bincount(idx, minlength=nd).astype(np.int64)
    plan = _plan(counts)
    ks = plan["ks"]
    cls = plan["cls"]
    nb = len(ks) + 2

    # ---- dom assignment (vectorized, shared structure) -------------------
    order = np.argsort(counts, kind="stable")
    n0 = int((counts == 0).sum())
    dom_core = np.full(nd, -1, np.int32)
    dom_win = np.zeros(nd, np.int64)
    dom_half = np.zeros(nd, np.int32)
    dom_dp = np.zeros(nd, np.int32)
    nfullwin = plan["nfullwin"]

    off = n0
    for k in ks:
        n, m, fw, r, rag_off_k, fullwin0 = cls[k]
        doms = order[off : off + n]
        off += n
        core = (np.arange(n) % NCORES).astype(np.int32)
        pos = (np.arange(n) // NCORES).astype(np.int64)
        isfull = pos < fw * 256
        rpos = rag_off_k + (pos - fw * 256)
        q = np.where(isfull, pos % 256, rpos % 256)
        win = np.where(isfull, fullwin0 + pos // 256, nfullwin + rpos // 256)
        dom_core[doms] = core
        dom_win[doms] = win
        dom_half[doms] = (q // 128).astype(np.int32)
        dom_dp[doms] = (q % 128).astype(np.int32)

    # ---- pulse permutation: sort by (core, window, half, dp) -------------
    dkey = (
        dom_core.astype(np.int64) * (1 << 40)
        + dom_win * 256
        + dom_half * 128
        + dom_dp
    )
    pkey = dkey[idx]
    perm = np.argsort(pkey, kind="stable")
    core_pulse_counts = np.bincount(dom_core[idx], minlength=NCORES)
    core_splits = np.concatenate([[0], np.cumsum(core_pulse_counts)])

    # ---- host-side slot buffer packing -----------------------------------
    E16 = E.astype(BF)
    s_elems = plan["s_elems"]
    RW = plan["RW"]

    # ragged group lookup per ragged window
    if RW:
        grp_of_rw = np.zeros(RW, np.int32)
        for gi, (win0_r, g, S, base) in enumerate(plan["rag_groups"]):
            grp_of_rw[win0_r : win0_r + g] = gi

    # full groups of each class, in order
    fg_of_k = {}
    for fg in plan["full_groups"]:
        fg_of_k.setdefault(fg[0], []).append(fg)

    e_idx = np.arange(D)
    bufs = []
    for c in range(NCORES):
        buf = np.zeros(s_elems, BF)
        pc = perm[core_splits[c] : core_splits[c + 1]]
        p = 0
        # full regions, class ascending
        for k in ks:
            n, m, fw, r, rag_off_k, fullwin0 = cls[k]
            nreal_c = n // NCORES + (1 if c < n % NCORES else 0)
            n_full = min(nreal_c, fw * 256)
            if fw == 0:
                continue
            if n_full == 0:
                continue
            rows = pc[p : p + n_full * k].reshape(n_full, k)
            p += n_full * k
            A = E16[rows]  # (n_full, k, 64)
            for (_, _, win0, g, S, base) in fg_of_k[k]:
                lo = (win0 - fullwin0) * 256
                hi = min(n_full, lo + g * 256)
                if hi <= lo:
                    continue
                T = buf[base : base + g * 128 * k * 128].reshape(g, 128, k, 128)
                nw_full = (hi - lo) // 256
                if nw_full:
                    B = (
                        A[lo : lo + nw_full * 256]
                        .reshape(nw_full, 2, 128, k, D)
                        .transpose(0, 1, 4, 3, 2)
                        .reshape(nw_full, 128, k, 128)
                    )
                    T[:nw_full] = B
                rem = (hi - lo) - nw_full * 256
                if rem:
                    Ar = A[lo + nw_full * 256 : hi]  # (rem, k, 64)
                    j = np.arange(rem)
                    h = (j // 128).astype(np.int32)
                    dp = (j % 128).astype(np.int32)
                    T[
                        nw_full,
                        (64 * h)[:, None, None] + e_idx[None, None, :],
                        np.arange(k)[None, :, None],
                        dp[:, None, None],
                    ] = Ar
        # ragged region, class ascending
        for k in ks:
            n, m, fw, r, rag_off_k, fullwin0 = cls[k]
            if r == 0:
                continue
            nreal_c = n // NCORES + (1 if c < n % NCORES else 0)
            n_full = min(nreal_c, fw * 256)
            n_rag = nreal_c - n_full
            if n_rag == 0:
                continue
            rows = pc[p : p + n_rag * k].reshape(n_rag, k)
            p += n_rag * k
            A = E16[rows]  # (n_rag, k, 64)
            rpos = rag_off_k + np.arange(n_rag)
            rw = rpos // 256
            grp = grp_of_rw[rw]
            bnds = np.concatenate(
                [[0], np.nonzero(np.diff(grp))[0] + 1, [n_rag]]
            )
            for bi in range(len(bnds) - 1):
                j0, j1 = int(bnds[bi]), int(bnds[bi + 1])
                gi = int(grp[j0])
                win0_r, g, S, base = plan["rag_groups"][gi]
                Ac = A[j0:j1]
                nn = j1 - j0
                Bf = np.empty((nn, S, D), BF)
                Bf[:, :k] = Ac
                padcnt = (S - 1) - k
                if padcnt > 0:
                    Bf[:, k : S - 1] = Ac[:, :1]
                Bf[:, S - 1] = (
                    -float(padcnt) * Ac[:, 0].astype(np.float32)
                ).astype(BF)
                T = buf[base : base + g * 128 * S * 128].reshape(g, 128, S, 128)
                rw_l = (rw[j0:j1] - win0_r).astype(np.int64)
                h = ((rpos[j0:j1] % 256) // 128).astype(np.int64)
                dp = (rpos[j0:j1] % 128).astype(np.int64)
                T[
                    rw_l[:, None, None],
                    (64 * h)[:, None, None] + e_idx[None, None, :],
                    np.arange(S)[None, :, None],
                    dp[:, None, None],
                ] = Bf
        bufs.append(buf)

    # ---- weights / bias / recip table ------------------------------------
    W1t = np.ascontiguousarray(W[:, :D].T)   # (64, 64) = W1.T
    W2t = np.ascontiguousarray(W[:, D:].T)
    wts = np.zeros((nb * 128, 128), np.float32)
    for j, k in enumerate(ks):
        blk = W1t / np.float32(k)
        wts[j * 128 : j * 128 + 64, :64] = blk
        wts[j * 128 + 64 : (j + 1) * 128, 64:] = blk
    j = len(ks)
    wts[j * 128 : j * 128 + 64, :64] = W1t
    wts[j * 128 + 64 : (j + 1) * 128, 64:] = W1t
    j = len(ks) + 1
    wts[j * 128 : j * 128 + 64, :64] = W2t
    wts[j * 128 + 64 : (j + 1) * 128, 64:] = W2t
    wts16 = wts.astype(BF)
    b2 = np.concatenate([b, b]).reshape(128, 1).astype(np.float32)

    rt = None
    if RW:
        karr = plan["karr"].astype(np.float64)
        recip = np.where(karr > 0, 1.0 / np.maximum(karr, 1), 0.0).astype(BF)
        rec = recip.reshape(RW, 2, 128)
        rt = np.zeros((128, RW * 128), BF)
        rt4 = rt.reshape(128, RW, 128)
        rt4[0:64] = rec[:, 0, :][None]
        rt4[64:128] = rec[:, 1, :][None]

    # ---- optional numpy emulation of the device program (KERNEL_SIM=1) ----
    if os.environ.get("KERNEL_SIM", "0") == "1":
        outs = np.stack(
            [_simulate(plan, bufs[c], wts, b2, rt, len(ks)) for c in range(NCORES)]
        )
        last_exec_ns = None
        return _unpermute(
            outs, plan, dom_core, dom_win, dom_half, dom_dp, nd, n0, b
        )

    # ---- device -----------------------------------------------------------
    nc = _build_nc(plan, nb)
    in_maps = []
    for c in range(NCORES):
        m = {"slots": bufs[c], "wts": wts16, "b": b2}
        if RW:
            m["rt"] = rt
        in_maps.append(m)
    trace = os.environ.get("KERNEL_TRACE", "0") == "1"
    kw_ = {}
    if trace and _install_trace_hook():
        import tempfile

        kw_ = dict(trace=True, tmpdir=tempfile.mkdtemp(prefix="kernel_trace_"))
    res = run_bass_kernel_spmd(nc, in_maps, core_ids=list(range(NCORES)), **kw_)
    last_exec_ns = res.exec_time_ns

    # ---- host-side unpermute ----------------------------------------------
    outs = np.stack(
        [np.asarray(res.results[c]["out"], dtype=np.float32) for c in range(NCORES)]
    )  # (8,128,ncols)
    return _unpermute(outs, plan, dom_core, dom_win, dom_half, dom_dp, nd, n0, b)


def _unpermute(outs, plan, dom_core, dom_win, dom_half, dom_dp, nd, n0, b):
    ncols = plan["nwin"] * 128
    outs_t = outs.transpose(0, 2, 1).reshape(NCORES, ncols, 2, D)
    full = np.empty((nd, D), np.float32)
    real = dom_core >= 0
    col = dom_win * 128 + dom_dp
    full[real] = outs_t[dom_core[real], col[real], dom_half[real]]
    if n0:
        full[~real] = b
    return full


def _simulate(plan, buf, wts, b2, rt, ncls):
    """Numpy emulation of the device program for one core (fp32 math)."""
    ncols = plan["nwin"] * 128
    out = np.zeros((128, ncols), np.float32)
    jW1un, jW2 = ncls, ncls + 1

    def blkT(j):
        return wts[j * 128 : (j + 1) * 128].astype(np.float32).T  # (128out,128in)

    for k, jcls, win0, g, S, base in plan["full_groups"]:
        T = buf[base : base + g * 128 * S * 128].reshape(g, 128, S, 128)
        Tf = T.astype(np.float32)
        ssum = Tf.sum(axis=2)          # (g, 128, 128)
        smax = Tf.max(axis=2)          # bf16-exact max
        for w in range(g):
            ps = blkT(jcls) @ ssum[w] + blkT(jW2) @ smax[w]
            out[:, (win0 + w) * 128 : (win0 + w + 1) * 128] = ps + b2
    for win0_r, g, S, base in plan["rag_groups"]:
        T = buf[base : base + g * 128 * S * 128].reshape(g, 128, S, 128)
        Tf = T.astype(np.float32)
        ssum = Tf.sum(axis=2)
        smax = Tf[:, :, : S - 1, :].max(axis=2)
        for w in range(g):
            rw = win0_r + w
            mean = (blkT(jW1un) @ ssum[w]) * rt[:, rw * 128 : (rw + 1) * 128]
            ps = mean + blkT(jW2) @ smax[w]
            out[:, (plan["nfullwin"] + rw) * 128 : (plan["nfullwin"] + rw + 1) * 128] = (
                ps + b2
            )
    return out
